# revision 34
# baseline (speedup 1.0000x reference)
"""Trainium2 Bass kernel for nn_CATCallerEncoderLayer (dynamic-conv encoder layer).

Reference computation (T=1024, B=16, C=512, H=8, K=31, P=15):
  h  = x @ w1 + b1; a, g = split(h); xg = a * sigmoid(g)
  w  = softmax((xg @ wl_w + wl_b).reshape(T,B,H,K), axis=-1)
  out[t,b,h*64+r] = sum_k w[t,b,h,k] * xg_pad[t+k-15, b, h*64+r]
  return out @ w2 + b2

Sharding: data-parallel over batch B across 8 cores (2 batches/core).

The PE (tensor-engine) queue is the bottleneck: runtime tracks PE
instruction count (416 matmuls + sync-split NoOps) times ~55us. Design:
  - f32r matmuls everywhere (1 instruction each; measured ~9ms faster
    in situ than plain f32, which walrus runs as slow 4-pass MMs)
  - consts loaded as f32r via gpsimd casting DMAs; xg staged once in
    bf16; conv halos cast bf16->f32r on gpsimd (Pool-queue waits are
    hidden under the PE shadow)
  - staging DMAs on the SP HW-DGE ring (FIFO elision merges waits)
  - PSUM in full-size waves (fewer accumulation-group boundaries beat
    double-buffered half waves: each group boundary costs a PE wait)
  - dynamic-conv banding via sheared DRAM M-form staging: feature-major
    softmax, token-major readback, stride-129 shear write, one batched
    band transpose per batch; single-DMA halos via the [head j|head j+4]
    chunking with overlapping zero-mask regions (w2 rows permuted
    host-side to match)
"""
import sys

sys.path.insert(0, "/opt/trn_rl_repo")

import numpy as np

T, B, C = 1024, 16, 512
H, KT, PAD = 8, 31, 15
HKP = 256            # padded hk: head h stripe at [32h, 32h+31)
NCORES = 8
BPC = B // NCORES    # 2
BT = 96              # conv time-block (s = tl + k <= 126 < 128)
NBLK = (T + BT - 1) // BT  # 11
SBLK = BT * 128      # staging elements per (b,i,h) block
NTC = T // 128       # 8

ZFILL = False
_cache = {}


def _split_sync_waits(nc, mybir, max_waits=1):
    """This walrus build rejects instructions carrying >1 sync-wait command.
    Hoist extra waits onto same-engine NOPs inserted just before."""
    cnt = 0
    for f in nc.m.functions:
        for bb in f.blocks:
            new = []
            for inst in bb.instructions:
                si = inst.sync_info
                if si is not None and si.on_wait and len(si.on_wait) > max_waits:
                    waits = list(si.on_wait)
                    for w in waits[:-max_waits]:
                        cnt += 1
                        new.append(
                            mybir.InstNoOp(
                                name=f"I-ws{cnt}",
                                engine=inst.engine,
                                sync_info=mybir.SyncInfo(on_wait=[w], on_update=[]),
                            )
                        )
                    inst.sync_info = mybir.SyncInfo(
                        on_wait=waits[-max_waits:], on_update=list(si.on_update or [])
                    )
                new.append(inst)
            bb.instructions = new
    return cnt


def _build(has_b1, has_wlb, has_b2, reps=1, zfill=None, phases='ADBM'):
    if zfill is None:
        zfill = ZFILL
    import bass_rust
    import concourse.bass as bass
    import concourse.tile as tile
    from concourse import mybir

    f32 = mybir.dt.float32
    f32r = mybir.dt.float32r
    bf16 = mybir.dt.bfloat16
    AF = mybir.ActivationFunctionType

    nc = bass.Bass("TRN2", debug=False)

    xt_d = nc.dram_tensor("xt", (BPC, C, T), f32, kind="ExternalInput").ap()
    w1_d = nc.dram_tensor("w1", (C, 2 * C), f32, kind="ExternalInput").ap()
    wlw_d = nc.dram_tensor("wl_w", (C, HKP), f32, kind="ExternalInput").ap()
    w2_d = nc.dram_tensor("w2", (C, C), f32, kind="ExternalInput").ap()
    sel_d = nc.dram_tensor("sel", (2, 128, 8), f32, kind="ExternalInput").ap()
    selt_d = nc.dram_tensor("selT", (2, 8, 128), f32, kind="ExternalInput").ap()
    b1_d = nc.dram_tensor("b1", (2 * C,), f32, kind="ExternalInput").ap()
    wlb_d = nc.dram_tensor("wl_b", (HKP,), f32, kind="ExternalInput").ap()
    b2_d = nc.dram_tensor("b2", (C,), f32, kind="ExternalInput").ap()
    out_d = nc.dram_tensor("out", (T, BPC, C), bf16, kind="ExternalOutput").ap()

    # scratch DRAM, double-buffered across reps so cross-rep WAR waits are
    # transitively elided (reps>1 exists only for slope timing; reps=1 uses
    # buffer 0 throughout). xgd has no margins: edge blocks use clipped
    # halo DMAs with partition memsets instead.
    xgdb2 = [nc.dram_tensor(f"xgdb{r}", (BPC * T, C), bf16).ap()
             for r in range(2)]
    stw2 = [nc.dram_tensor(f"stw{r}", (BPC * HKP, T), bf16).ap()
            for r in range(2)]
    recd2 = [nc.dram_tensor(f"recd{r}", (BPC * 2 * 8, 512), f32).ap()
             for r in range(2)]
    stm2 = [nc.dram_tensor("stm", (BPC * NBLK * H * BT, 128), bf16,
                           kind="ExternalInput").ap(),
            nc.dram_tensor("stm2", (BPC * NBLK * H * BT, 128), bf16,
                           kind="ExternalInput").ap()]
    stm_fl = [x[:].flatten() for x in stm2]
    xgdb_fl = [x[:].flatten() for x in xgdb2]
    stw_fl = [x[:].flatten() for x in stw2]
    xt_f = xt_d[:].flatten()
    w1_f = w1_d[:].flatten()
    w2_f = w2_d[:].flatten()
    wlw_f = wlw_d[:].flatten()
    sel_f = sel_d[:].flatten()
    selt_f = selt_d[:].flatten()
    wlb_f = wlb_d[:].flatten()
    out_f = out_d[:].flatten()

    def ap_of(base, dims, offset):
        a = base[:1].copy()
        a.ap = bass_rust.VecI64Pair(dims)
        a.offset = offset
        return a

    with tile.TileContext(nc) as tc:
        with (
            tc.tile_pool(name="consts", bufs=1) as cpool,
            tc.tile_pool(name="xgbp", bufs=1) as xgbpool,
            tc.tile_pool(name="glup", bufs=1) as glupool,
            tc.tile_pool(name="dynp", bufs=1) as dynpool,
            tc.tile_pool(name="xgfp", bufs=1) as xgfpool,
            tc.tile_pool(name="wtp", bufs=2) as wtpool,
            tc.tile_pool(name="convp", bufs=1) as convpool,
            tc.tile_pool(name="bandp", bufs=1) as bandpool,
            tc.tile_pool(name="osp", bufs=1) as ospool,
        ):
            # ---- constants (all plain f32 on the SP ring) ----
            xtt = []
            for b in range(BPC):
                tx = cpool.tile([128, 4, T], f32r, tag=f"xt{b}", name=f"xt{b}")
                src = ap_of(xt_f,
                            [[T, 128], [128 * T, 4], [1, T]], b * C * T)
                nc.gpsimd.dma_start(tx[:], src)
                xtt.append(tx)
            w1t = cpool.tile([128, 4, 2 * C], f32r, tag="w1t")
            nc.gpsimd.dma_start(
                w1t[:],
                ap_of(w1_f,
                      [[2 * C, 128], [128 * 2 * C, 4], [1, 2 * C]], 0))
            w2t = cpool.tile([128, 4, C], f32r, tag="w2t")
            nc.gpsimd.dma_start(
                w2t[:],
                ap_of(w2_f,
                      [[C, 128], [128 * C, 4], [1, C]], 0))
            wlwt = cpool.tile([128, 4, HKP], f32r, tag="wlwt")
            nc.gpsimd.dma_start(
                wlwt[:],
                ap_of(wlw_f,
                      [[HKP, 128], [128 * HKP, 4], [1, HKP]], 0))
            selt_t = cpool.tile([128, 2, 8], f32r, tag="sel")
            nc.gpsimd.dma_start(
                selt_t[:],
                ap_of(sel_f, [[8, 128], [128 * 8, 2], [1, 8]], 0))
            selTt = cpool.tile([8, 2, 128], f32r, tag="selT")
            nc.gpsimd.dma_start(
                selTt[:],
                ap_of(selt_f, [[128, 8], [8 * 128, 2], [1, 128]], 0))
            if has_b1:
                b1t = cpool.tile([128, 2, C], f32, tag="b1t")
                nc.sync.dma_start(
                    b1t[:], b1_d[None, :].to_broadcast((128, 2 * C)).rearrange(
                        "p (g c) -> p g c", c=C))
            if has_wlb:
                # per-partition bias for feature-major wT chunks [128, 1]
                wlbt = cpool.tile([128, 2], f32, tag="wlbt")
                nc.sync.dma_start(
                    wlbt[:],
                    ap_of(wlb_f, [[1, 128], [128, 2]], 0))
            if has_b2:
                b2t = cpool.tile([128, C], f32, tag="b2t")
                nc.sync.dma_start(b2t[:], b2_d[None, :].to_broadcast((128, C)))

            # zero-fill xgd margins (f32 zeros)
            ztf = cpool.tile([128, 256], f32, tag="ztf")
            nc.vector.memset(ztf[:], 0.0)
            ztr = ztf[:].bitcast(f32r)
            # combined masked halo tile: A-slot j = cols [128j,128j+128) =
            # [head-j data | zeros]; B-slot j = cols [448+128j, 448+128j+128)
            # = [zeros | head-(j+4) data]. Zero regions coincide on [448,512).
            # Data halves of all 8 slots form one affine pattern with the
            # source being contiguous natural-order channels -> 1 DMA/block.
            halo2s = []
            for hb in range(NBLK):
                h2 = cpool.tile([128, 960], f32r, tag=f"halo2_{hb}",
                                name=f"halo2_{hb}")
                nc.vector.tensor_copy(
                    h2[:, 0:512].rearrange("p (j c) -> p j c", c=128)[:, :, 64:128],
                    ztr[:, 0:256].rearrange("p (j c) -> p j c", c=64))
                nc.vector.tensor_copy(
                    h2[:, 448:960].rearrange("p (j c) -> p j c", c=128)[:, :, 0:64],
                    ztr[:, 0:256].rearrange("p (j c) -> p j c", c=64))
                halo2s.append(h2)
            if zfill:
                ztb = cpool.tile([128, 4096], bf16, tag="ztb")
                nc.vector.memset(ztb[:], 0.0)
                nrows = BPC * NBLK * H * BT
                for sb in stm2:
                    pos = 0
                    while pos < nrows:
                        n = min(4096, nrows - pos)
                        nc.sync.dma_start(sb[pos:pos + n, :], ztb[:, :n])
                        pos += n
            del zfill

            for rep in range(reps):
                xgdb_f = xgdb_fl[rep % 2]
                recd_t = recd2[rep % 2]
                stw_f = stw_fl[rep % 2]
                stm_f = stm_fl[rep % 2]
                # ============ Phase A: mm1 + GLU (8-bank PSUM waves) ============
                with tc.tile_pool(name=f"psA{rep}", bufs=1,
                                  space="PSUM") as psA:
                  for b in range(BPC if 'A' in phases else 0):
                    xgbs = []
                    if True:
                        for wave in range(2):
                            xgb = xgbpool.tile([128, 4, C], bf16, tag="xgb",
                                               name=f"xgb{rep}_{b}_{wave}")
                            xgbs.append(xgb)
                            h1a = psA.tile([128, 4, C], f32, tag="h1a")
                            h1g = psA.tile([128, 4, C], f32, tag="h1g")
                            for tt in range(4):
                                t0 = wave * 512 + tt * 128
                                for cc in range(4):
                                    nc.tensor.matmul(
                                        h1a[:, tt, :], xtt[b][:, cc, t0:t0 + 128],
                                        w1t[:, cc, 0:C],
                                        start=(cc == 0), stop=(cc == 3))
                                for cc in range(4):
                                    nc.tensor.matmul(
                                        h1g[:, tt, :], xtt[b][:, cc, t0:t0 + 128],
                                        w1t[:, cc, C:2 * C],
                                        start=(cc == 0), stop=(cc == 3))
                            sg = glupool.tile([128, 4, C], f32, tag="sg")
                            if has_b1:
                                gb = glupool.tile([128, 4, C], f32, tag="gb")
                                nc.vector.tensor_add(
                                    gb[:], h1g[:],
                                    b1t[:, None, 1, :].to_broadcast((128, 4, C)))
                                nc.scalar.activation(sg[:], gb[:], AF.Sigmoid)
                                ab = glupool.tile([128, 4, C], f32, tag="ab")
                                nc.vector.tensor_add(
                                    ab[:], h1a[:],
                                    b1t[:, None, 0, :].to_broadcast((128, 4, C)))
                                nc.vector.tensor_mul(xgb[:], ab[:], sg[:])
                            else:
                                nc.scalar.activation(sg[:], h1g[:], AF.Sigmoid)
                                nc.vector.tensor_mul(xgb[:], h1a[:], sg[:])
                            # token-major bf16 store per wave
                            off = (b * T + wave * 512) * C
                            dstb = ap_of(xgdb_f,
                                         [[C, 128], [128 * C, 4], [1, C]], off)
                            nc.sync.dma_start(dstb, xgb[:])

                # ============ Phase dyn: wT = wlw.T @ xg.T, softmax ============
                with tc.tile_pool(name=f"psD{rep}", bufs=1, space="PSUM") as psD, \
                     tc.tile_pool(name=f"psDs{rep}", bufs=1, space="PSUM") as psDs:
                    for b in range(BPC if 'D' in phases else 0):
                        wT2 = psD.tile([128, 2, 2, 512], f32, tag="wT2")
                        for cc in range(4):
                            txb = glupool.tile([128, T], bf16, tag="xgfb")
                            src = ap_of(xgdb_f, [[C, T], [1, 128]],
                                        b * T * C + cc * 128)
                            nc.sync.dma_start(txb[:], src, transpose=True)
                            tx = xgfpool.tile([128, T], f32r, tag="xgf")
                            nc.vector.tensor_copy(tx[:], txb[:])
                            for tb in range(2):
                                for m in range(2):
                                    nc.tensor.matmul(
                                        wT2[:, tb, m, :],
                                        wlwt[:, cc, m * 128:(m + 1) * 128],
                                        tx[:, tb * 512:tb * 512 + 512],
                                        start=(cc == 0), stop=(cc == 3))
                        we2 = dynpool.tile([128, 2, 2, 512], f32r, tag="we2")
                        if has_wlb:
                            for tb in range(2):
                                for m in range(2):
                                    nc.scalar.activation(
                                        we2[:, tb, m, :], wT2[:, tb, m, :],
                                        AF.Exp, bias=wlbt[:, m:m + 1])
                        else:
                            nc.scalar.activation(we2[:], wT2[:], AF.Exp)
                        for tb in range(2):
                            ts0 = tb * 512
                            sums = psDs.tile([8, 512], f32, tag="sums")
                            nc.tensor.matmul(sums[:], selt_t[:, 0, :],
                                             we2[:, tb, 0, :],
                                             start=True, stop=False)
                            nc.tensor.matmul(sums[:], selt_t[:, 1, :],
                                             we2[:, tb, 1, :],
                                             start=False, stop=True)
                            rec = dynpool.tile([8, 512], f32, tag="rec")
                            nc.vector.reciprocal(rec[:], sums[:])
                            # broadcast 1/sum over the 32 k-partitions per
                            # head via DRAM roundtrip + per-head to_broadcast
                            # reads (proven b1-style form) -- off the PE queue
                            r0 = (b * 2 + tb) * 8
                            nc.sync.dma_start(recd_t[r0:r0 + 8, :], rec[:])
                            rrepb = dynpool.tile([128, 2, 512], f32,
                                                 tag="rrepb")
                            for m in range(2):
                                for hh in range(4):
                                    src = recd_t[r0 + 4 * m + hh:
                                                 r0 + 4 * m + hh + 1,
                                                 :].to_broadcast((32, 512))
                                    nc.sync.dma_start(
                                        rrepb[32 * hh:32 * hh + 32, m, :], src)
                            wsb = dynpool.tile([128, 2, 512], bf16, tag="wsb")
                            nc.vector.tensor_mul(
                                wsb[:], we2[:, tb, :, :].bitcast(f32),
                                rrepb[:])
                            dst = ap_of(stw_f,
                                        [[T, 128], [128 * T, 2], [1, 512]],
                                        b * HKP * T + ts0)
                            nc.sync.dma_start(dst, wsb[:])

                # token-major w readback + M-form shear-write
                for b in range(BPC if 'D' in phases else 0):
                    for tch in range(NTC):
                        t0 = tch * 128
                        wt = wtpool.tile([128, HKP], bf16, tag="wt")
                        src = ap_of(stw_f, [[T, HKP], [1, 128]], b * HKP * T + t0)
                        nc.sync.dma_start(wt[:], src, transpose=True)
                        i = t0 // BT
                        while i < NBLK and i * BT < t0 + 128:
                            r0 = max(t0, i * BT)
                            r1 = min(t0 + 128, i * BT + BT, T)
                            nr = r1 - r0
                            tl0 = r0 - i * BT
                            src3 = wt[r0 - t0:r1 - t0, :].rearrange(
                                "t (h k) -> t h k", k=32)[:, :, 0:KT]
                            dst = ap_of(stm_f, [[129, nr], [SBLK, H], [1, KT]],
                                        ((b * NBLK + i) * H) * SBLK + tl0 * 129)
                            nc.sync.dma_start(dst, src3)
                            i += 1

                # ============ Phase B: banded conv ============
                with tc.tile_pool(name=f"psB{rep}", bufs=2,
                                  space="PSUM") as psB, \
                     tc.tile_pool(name=f"psC{rep}", bufs=1,
                                  space="PSUM") as psC:
                  for b in range(BPC if 'B' in phases else 0):
                    ca = convpool.tile([128, 4, T], f32r, tag="convA")
                    bandall = bandpool.tile([128, NBLK * H * BT], bf16,
                                            tag="bandall", name=f"ba{rep}{b}")
                    hseg = 6 * H * BT
                    src = ap_of(stm_f, [[128, hseg], [1, 128]],
                                b * NBLK * H * SBLK)
                    nc.sync.dma_start(bandall[:, 0:hseg], src, transpose=True)
                    src = ap_of(stm_f, [[128, NBLK * H * BT - hseg], [1, 128]],
                                b * NBLK * H * SBLK + hseg * 128)
                    nc.sync.dma_start(bandall[:, hseg:], src, transpose=True)
                    bandf = bandpool.tile([128, NBLK * H * BT], f32r,
                                          tag="bandf", name=f"bf{rep}{b}")
                    nc.vector.tensor_copy(bandf[:], bandall[:])
                    if True:
                        for ip in range((NBLK + 1) // 2):
                            cp = psB.tile([128, 2, 4, 128], f32, tag="cp")
                            t0p = ip * 2 * BT
                            npair = min(2, NBLK - ip * 2)
                            for g in range(npair):
                                i = ip * 2 + g
                                t0 = i * BT
                                blk = min(BT, T - t0)
                                t0 = i * BT
                                lo = t0 - PAD
                                halo2 = halo2s[i]
                                p0 = max(0, -lo)
                                p1 = min(128, T - lo)
                                if p0 > 0:
                                    nc.vector.memset(
                                        halo2[0:(p0 + 31) // 32 * 32,
                                              :].bitcast(f32), 0.0)
                                if p1 < 128:
                                    nc.vector.memset(
                                        halo2[p1 // 32 * 32:128,
                                              :].bitcast(f32), 0.0)
                                hd = halo2[p0:p1, :].copy()
                                hd.ap = bass_rust.VecI64Pair(
                                    [[960, p1 - p0], [128, 8], [1, 64]])
                                src = ap_of(xgdb_f, [[C, p1 - p0], [1, 512]],
                                            (b * T + lo + p0) * C)
                                nc.gpsimd.dma_start(hd, src)
                                i0 = i * H * BT
                                for j in range(4):
                                    nc.tensor.matmul(
                                        cp[:, g, j, 0:blk],
                                        halo2[:, 128 * j:128 * j + 128],
                                        bandf[:, i0 + j * BT:i0 + j * BT + blk],
                                        start=True, stop=False)
                                    nc.tensor.matmul(
                                        cp[:, g, j, 0:blk],
                                        halo2[:, 448 + 128 * j:576 + 128 * j],
                                        bandf[:, i0 + (j + 4) * BT:
                                              i0 + (j + 4) * BT + blk],
                                        start=False, stop=True)
                            span = min(npair * BT, T - t0p)
                            w = BT if npair == 2 else span
                            dv = ca[:, :, t0p:t0p + span].rearrange(
                                "p j (g t) -> p g j t", t=w)
                            nc.vector.tensor_copy(dv, cp[:, 0:npair, :, 0:w])

                    # ============ mm2 for this batch ============
                    if True:
                        for wave in range(2 if 'M' in phases else 0):
                            op = psC.tile([128, 4, C], f32, tag="op")
                            for tt in range(4):
                                t0 = wave * 512 + tt * 128
                                for j in range(4):
                                    nc.tensor.matmul(
                                        op[:, tt, :],
                                        ca[:, j, t0:t0 + 128],
                                        w2t[:, j, :],
                                        start=(j == 0), stop=(j == 3))
                            os = ospool.tile([128, 4, C], bf16, tag="os")
                            if has_b2:
                                nc.vector.tensor_add(
                                    os[:], op[:],
                                    b2t[:, None, :].to_broadcast((128, 4, C)))
                            else:
                                nc.scalar.copy(os[:], op[:])
                            dst = ap_of(out_f,
                                        [[BPC * C, 128], [128 * BPC * C, 4],
                                         [1, C]],
                                        (wave * 512 * BPC + b) * C)
                            nc.sync.dma_start(dst, os[:])

    _split_sync_waits(nc, mybir)
    return nc


def _prep_w2(w2):
    # conv output chunk j rows = [head j (64) | head j+4 (64)]
    perm = np.empty((C,), np.int64)
    for j in range(4):
        perm[128 * j:128 * j + 64] = np.arange(64) + 64 * j
        perm[128 * j + 64:128 * j + 128] = np.arange(64) + 64 * (j + 4)
    return np.ascontiguousarray(w2[perm, :])


def kernel(x, w1, b1, wl_w, wl_b, w2, b2):
    from concourse.bass_utils import run_bass_kernel_spmd

    import ml_dtypes
    x = np.asarray(x, np.float32)
    w1 = np.asarray(w1, np.float32)
    b1 = np.asarray(b1, np.float32)
    wl_w = np.asarray(wl_w, np.float32)
    wl_b = np.asarray(wl_b, np.float32)
    w2 = np.asarray(w2, np.float32)
    b2 = np.asarray(b2, np.float32)

    has_b1 = bool(np.any(b1))
    has_wlb = bool(np.any(wl_b))
    has_b2 = bool(np.any(b2))

    key = (has_b1, has_wlb, has_b2)
    if key not in _cache:
        _cache[key] = _build(*key)
    nc = _cache[key]

    # restripe wl_w / wl_b to padded 32-per-head layout
    wlw_pad = np.zeros((C, HKP), np.float32)
    wlb_pad = np.zeros((HKP,), np.float32)
    for h in range(H):
        wlw_pad[:, 32 * h:32 * h + KT] = wl_w[:, KT * h:KT * (h + 1)]
        wlb_pad[32 * h:32 * h + KT] = wl_b[KT * h:KT * (h + 1)]
    # sel: [chunk m, p, h] -> 1 where head(m*128+p) == h and k < 31
    sel = np.zeros((2, 128, 8), np.float32)
    selT = np.zeros((2, 8, 128), np.float32)
    for m in range(2):
        for p in range(128):
            hk = m * 128 + p
            h, k = hk // 32, hk % 32
            if k < KT:
                sel[m, p, h] = 1.0
            selT[m, h, p] = 1.0

    in_maps = []
    for c in range(NCORES):
        xs = x[:, c * BPC:(c + 1) * BPC, :]  # (T, BPC, C)
        xt = np.ascontiguousarray(xs.transpose(1, 2, 0)).astype(np.float32)
        in_maps.append({
            "xt": xt, "w1": w1, "wl_w": wlw_pad, "w2": _prep_w2(w2),
            "b1": b1, "wl_b": wlb_pad, "b2": b2,
            "sel": sel, "selT": selT,
            "stm": np.zeros((BPC * NBLK * H * BT, 128), dtype=ml_dtypes.bfloat16),
            "stm2": np.zeros((BPC * NBLK * H * BT, 128),
                             dtype=ml_dtypes.bfloat16),
        })

    res = run_bass_kernel_spmd(nc, in_maps, core_ids=list(range(NCORES)))
    out = np.empty((T, B, C), np.float32)
    for c in range(NCORES):
        out[:, c * BPC:(c + 1) * BPC, :] = np.asarray(
            res.results[c]["out"], dtype=np.float32)
    return out
